# revision 4
# baseline (speedup 1.0000x reference)
"""F1-score (macro) kernel for Trainium2, 8 NeuronCores.

Data-parallel over rows (125000/core). Per tile of 2048 rows
([128p, TK=16, 128c], row = base + p*TK + k):

  - DVE : rowmax via pairwise-max tree. Step 1 reads the HIGH bf16 halves
          of the f32 tile (bitcast stride-2 view) -> truncated-bf16 max
          (trunc <= true max, so the argmax position always compares as
          "not less"). Steps 2..4 run at DVE 2x on packed bf16; final
          8-wide reduce emits f32.
  - DVE : oht_ck[p,c,k] = (c == y_true) in ck-layout - a single is_eq at
          DVE 2x (broadcasting t along the MIDDLE dim keeps every
          operand's last dim packed bf16).
  - anti split: first NDVE chunks as ONE sub-tile tensor_tensor is_lt on
          DVE ({0,1}); ~half the remaining on ACT via Sign ({-1,+1}!);
          one trailing chunk on GpSimd every other tile ({0,1}).
  - PE  : acc[bank] += oht_ck[:,:,k]^T @ anti[:,k,:] (bf16, 4 PSUM banks:
          banks 0/1 collect {0,1}-chunks, banks 2/3 the {-1,+1} chunks).

Host: with per-group supports S01/S23 (row->engine map is static):
  pred01 = S01 - (bank0+bank1),  pred23 = (S23 - (bank2+bank3)) / 2
  cm = pred01 + pred23;  macro-F1 epilogue on [128,128].

bf16 truncation ties perturb F1 by ~2.7e-4 (measured in numpy on the
actual inputs), far under the 2e-2 gate.
"""

import sys
import time

if "/opt/trn_rl_repo" not in sys.path:
    sys.path.insert(0, "/opt/trn_rl_repo")

import numpy as np

import concourse.bacc as bacc
import concourse.mybir as mybir
import concourse.tile as tile
from concourse import bass_utils

C = 128
N = 1_000_000
NCORES = 8
R = N // NCORES          # 125000 rows per core
TK = 16                  # chunks (of 128 rows) per tile
TR = 128 * TK            # 2048 rows per tile
NT = R // TR             # 61 tiles
TAIL = R - NT * TR       # 72 rows
EPS = 1e-12

NDVE = 3                 # max leading anti chunks per tile on DVE ({0,1})


def _gs_chunks(i):
    """GpSimd-owned trailing anti chunks ({0,1}) for tile i."""
    return (15,)


def _ndve(i):
    """Leading anti chunks on DVE for tile i."""
    return 3 if i % 2 == 0 else 2


def _schedule():
    """(i, k) -> (engine, bank); engine in {'dve','act','gs'}."""
    sched = {}
    nb01 = 0
    nb23 = 0
    for i in range(NT):
        gs = _gs_chunks(i)
        nd = _ndve(i)
        for k in range(TK):
            if k < nd:
                sched[(i, k)] = ("dve", nb01 % 4)
                nb01 += 1
            elif k in gs:
                sched[(i, k)] = ("gs", nb01 % 4)
                nb01 += 1
            else:
                sched[(i, k)] = ("act", 4 + nb23 % 4)
                nb23 += 1
    return sched


_SCHED = _schedule()
# last matmul per bank 1..7 (bank 0 ends with the tail matmul)
_LAST = {}
for (_i, _k), (_e, _b) in _SCHED.items():
    _LAST[_b] = max(_LAST.get(_b, (-1, -1)), (_i, _k))

_CACHE = {}


def _build():
    f32 = mybir.dt.float32
    bf16 = mybir.dt.bfloat16
    Alu = mybir.AluOpType
    Act = mybir.ActivationFunctionType

    nc = bacc.Bacc("TRN2", target_bir_lowering=False, debug=False,
                   num_devices=NCORES)
    yp = nc.dram_tensor("yp", [R, C], f32, kind="ExternalInput")
    yt = nc.dram_tensor("yt", [R], bf16, kind="ExternalInput")
    cm4 = nc.dram_tensor("cm4", [C, 8, C], f32, kind="ExternalOutput")

    with tile.TileContext(nc) as tc:
        with (
            tc.tile_pool(name="const", bufs=1) as cpool,
            tc.tile_pool(name="xin", bufs=4) as xpool,
            tc.tile_pool(name="oh", bufs=5) as ohpool,
            tc.tile_pool(name="an", bufs=5) as anpool,
            tc.tile_pool(name="tree", bufs=4) as tpool,
            tc.tile_pool(name="small", bufs=6) as spool,
            tc.tile_pool(name="psum", bufs=1, space="PSUM") as psum,
        ):
            iota_i = cpool.tile([128, C], mybir.dt.int32)
            nc.gpsimd.iota(iota_i[:], pattern=[[1, C]], base=0,
                           channel_multiplier=0)
            iota_bf = cpool.tile([128, C], bf16)
            nc.vector.tensor_copy(iota_bf[:], iota_i[:])
            # iota_ck[p, c, k] = c  (constant along inner k)
            iota_ck = cpool.tile([128, C, TK], bf16)
            nc.vector.tensor_copy(
                iota_ck[:], iota_bf[:, :, None].broadcast_to([128, C, TK])
            )

            accs = [
                psum.tile([C, C], f32, tag=f"acc{j}", name=f"acc{j}")
                for j in range(8)
            ]
            started = [False] * 8

            def mm(bank, lhsT, rhs, is_last=False):
                nc.tensor.matmul(
                    accs[bank][:], lhsT, rhs,
                    start=not started[bank], stop=is_last,
                )
                started[bank] = True

            def emit_tile(i):
                base = i * TR
                x = xpool.tile([128, TK, C], f32, tag="x", name="x")
                nc.sync.dma_start(
                    x[:],
                    yp.ap()[base : base + TR, :].rearrange(
                        "(p k) c -> p k c", k=TK
                    ),
                )
                t = spool.tile([128, TK], bf16, tag="t", name="t")
                nc.sync.dma_start(
                    t[:],
                    yt.ap()[base : base + TR].rearrange("(p k) -> p k", k=TK),
                )

                # one-hot(true) in ck layout: single 2x is_eq
                oht = ohpool.tile([128, C, TK], bf16, tag="oht", name="oht")
                nc.vector.tensor_tensor(
                    oht[:], iota_ck[:],
                    t[:, None, :].broadcast_to([128, C, TK]),
                    op=Alu.is_equal,
                )

                # rowmax tree; step 1 over high bf16 halves (truncation)
                xh = x.bitcast(bf16).rearrange(
                    "p k (c two) -> p k c two", two=2
                )[:, :, :, 1]
                m1 = tpool.tile([128, TK, 64], bf16, tag="m1", name="m1")
                nc.vector.tensor_tensor(
                    m1[:], xh[:, :, 0:64], xh[:, :, 64:128], op=Alu.max
                )
                m2 = tpool.tile([128, TK, 32], bf16, tag="m2", name="m2")
                nc.vector.tensor_tensor(
                    m2[:], m1[:, :, 0:32], m1[:, :, 32:64], op=Alu.max
                )
                m3 = tpool.tile([128, TK, 16], bf16, tag="m3", name="m3")
                nc.vector.tensor_tensor(
                    m3[:], m2[:, :, 0:16], m2[:, :, 16:32], op=Alu.max
                )
                m4 = tpool.tile([128, TK, 8], bf16, tag="m4", name="m4")
                nc.vector.tensor_tensor(
                    m4[:], m3[:, :, 0:8], m3[:, :, 8:16], op=Alu.max
                )
                rmax = spool.tile([128, TK], f32, tag="rmax", name="rmax")
                nc.vector.tensor_reduce(
                    rmax[:], m4[:], axis=mybir.AxisListType.X, op=Alu.max
                )

                # anti chunks
                anti = anpool.tile([128, TK, C], bf16, tag="anti",
                                   name="anti")
                nd = _ndve(i)
                nc.vector.tensor_tensor(
                    anti[:, 0:nd, :], x[:, 0:nd, :],
                    rmax[:, 0:nd, None].broadcast_to([128, nd, C]),
                    op=Alu.is_lt,
                )
                gs = _gs_chunks(i)
                for k in range(nd, TK):
                    if k in gs:
                        nc.gpsimd.tensor_scalar(
                            anti[:, k, :], x[:, k, :], rmax[:, k : k + 1],
                            None, op0=Alu.is_lt,
                        )
                    else:
                        nc.scalar.activation(
                            anti[:, k, :], x[:, k, :], Act.Sign,
                            bias=rmax[:, k : k + 1], scale=-1.0,
                        )

                for k in range(TK):
                    eng, bank = _SCHED[(i, k)]
                    mm(bank, anti[:, k, :], oht[:, :, k],
                       is_last=(bank != 0 and _LAST[bank] == (i, k)))

            for i in range(NT):
                emit_tile(i)

            # tail rows (72): {0,1} semantics -> bank 0
            base = NT * TR
            xt = xpool.tile([TAIL, 1, C], f32, tag="xtail", name="xt")
            nc.sync.dma_start(
                xt[:],
                yp.ap()[base : R, :].rearrange("(p k) c -> p k c", k=1),
            )
            tt = spool.tile([TAIL, 1], bf16, tag="ttail", name="tt")
            nc.sync.dma_start(
                tt[:], yt.ap()[base : R].rearrange("(p k) -> p k", k=1)
            )
            tt_f = spool.tile([TAIL, 1], f32, tag="ttailf", name="tt_f")
            nc.vector.tensor_copy(tt_f[:], tt[:])
            xht = xt.bitcast(bf16).rearrange(
                "p k (c two) -> p k c two", two=2
            )[:, :, :, 1]
            rmax_tb = spool.tile([TAIL, 1], bf16, tag="rmaxtb", name="rmax_tb")
            nc.vector.tensor_reduce(
                rmax_tb[:], xht[:], axis=mybir.AxisListType.X, op=Alu.max
            )
            rmax_t = spool.tile([TAIL, 1], f32, tag="rmaxtail", name="rmax_t")
            nc.vector.tensor_copy(rmax_t[:], rmax_tb[:])
            anti_t = anpool.tile([TAIL, C], bf16, tag="antitail",
                                 name="anti_t")
            oht_t = ohpool.tile([TAIL, C], bf16, tag="ohttail", name="oht_t")
            nc.vector.tensor_scalar(
                anti_t[:], xt[:, 0, :], rmax_t[:], None, op0=Alu.is_lt
            )
            nc.vector.tensor_scalar(
                oht_t[:], iota_bf[:TAIL, :], tt_f[:], None, op0=Alu.is_equal
            )
            nc.tensor.matmul(
                accs[0][:], anti_t[:], oht_t[:], start=False, stop=True
            )

            out_sb = cpool.tile([C, 8, C], f32)
            for j in range(8):
                nc.scalar.copy(out_sb[:, j, :], accs[j][:])
            nc.sync.dma_start(cm4.ap()[:], out_sb[:])

    nc.compile()
    return nc


def _group01_mask():
    """Per-core mask over R rows: True if the row's anti chunk has {0,1}
    semantics (DVE/GpSimd/tail), False for ACT ({-1,+1})."""
    r = np.arange(R)
    i = r // TR
    k = r % TK
    m = i == NT  # tail rows
    for it in range(NT):
        sel = i == it
        m |= sel & (k < _ndve(it))
        gs = _gs_chunks(it)
        if gs:
            m |= sel & np.isin(k, gs)
    return m


_G01 = _group01_mask()


def _get_nc():
    if "nc" not in _CACHE:
        _CACHE["nc"] = _build()
    return _CACHE["nc"]


def _run(y_pred, y_true, trace=False):
    import ml_dtypes

    nc = _get_nc()
    y_pred = np.ascontiguousarray(np.asarray(y_pred, dtype=np.float32))
    yt_i = np.asarray(y_true).astype(np.int64)
    yt_bf = yt_i.astype(ml_dtypes.bfloat16)
    in_maps = [
        {
            "yp": y_pred[c * R : (c + 1) * R],
            "yt": np.ascontiguousarray(yt_bf[c * R : (c + 1) * R]),
        }
        for c in range(NCORES)
    ]
    res = None
    for attempt in range(3):
        try:
            res = bass_utils.run_bass_kernel_spmd(
                nc, in_maps, core_ids=list(range(NCORES)), trace=trace
            )
            break
        except Exception:
            if attempt == 2:
                raise
            time.sleep(2.0)
    b01 = np.zeros((C, C), dtype=np.float64)
    b23 = np.zeros((C, C), dtype=np.float64)
    s01 = np.zeros(C, dtype=np.float64)
    s23 = np.zeros(C, dtype=np.float64)
    for c, r in enumerate(res.results):
        cm_b = r["cm4"].astype(np.float64)
        # banks hold cm^T (anti as stationary operand): transpose back
        b01 += cm_b[:, 0:4, :].sum(axis=1).T
        b23 += cm_b[:, 4:8, :].sum(axis=1).T
        yt_c = yt_i[c * R : (c + 1) * R]
        s01 += np.bincount(yt_c[_G01], minlength=C)
        s23 += np.bincount(yt_c[~_G01], minlength=C)
    cm = (s01[:, None] - b01) + (s23[:, None] - b23) / 2.0
    diag = np.diagonal(cm)
    precision = diag / (cm.sum(axis=1) + EPS)
    recall = diag / (cm.sum(axis=0) + EPS)
    f1 = 2.0 * precision * recall / (precision + recall + EPS)
    return np.float32(f1.mean()), res


def kernel(y_pred, y_true):
    out, _ = _run(y_pred, y_true, trace=False)
    return out


# revision 5
# speedup vs baseline: 1.0174x; 1.0174x over previous
"""F1-score (macro) kernel for Trainium2, 8 NeuronCores.

Data-parallel over rows (125000/core). Per tile of 2048 rows
([128p, TK=16, 128c], row = base + p*TK + k):

  - DVE : rowmax via pairwise-max tree. Step 1 reads the HIGH bf16 halves
          of the f32 tile (bitcast stride-2 view) -> truncated-bf16 max
          (trunc <= true max, so the argmax position always compares as
          "not less"). Steps 2..4 run at DVE 2x on packed bf16; final
          8-wide reduce emits f32.
  - DVE : oht_ck[p,c,k] = (c == y_true) in ck-layout - a single is_eq at
          DVE 2x (broadcasting t along the MIDDLE dim keeps every
          operand's last dim packed bf16).
  - anti split: first NDVE chunks as ONE sub-tile tensor_tensor is_lt on
          DVE ({0,1}); ~half the remaining on ACT via Sign ({-1,+1}!);
          one trailing chunk on GpSimd every other tile ({0,1}).
  - PE  : acc[bank] += oht_ck[:,:,k]^T @ anti[:,k,:] (bf16, 4 PSUM banks:
          banks 0/1 collect {0,1}-chunks, banks 2/3 the {-1,+1} chunks).

Host: with per-group supports S01/S23 (row->engine map is static):
  pred01 = S01 - (bank0+bank1),  pred23 = (S23 - (bank2+bank3)) / 2
  cm = pred01 + pred23;  macro-F1 epilogue on [128,128].

bf16 truncation ties perturb F1 by ~2.7e-4 (measured in numpy on the
actual inputs), far under the 2e-2 gate.
"""

import sys
import time

if "/opt/trn_rl_repo" not in sys.path:
    sys.path.insert(0, "/opt/trn_rl_repo")

import numpy as np

import concourse.bacc as bacc
import concourse.mybir as mybir
import concourse.tile as tile
from concourse import bass_utils

C = 128
N = 1_000_000
NCORES = 8
R = N // NCORES          # 125000 rows per core
TK = 16                  # chunks (of 128 rows) per tile
TR = 128 * TK            # 2048 rows per tile
NT = R // TR             # 61 tiles
TAIL = R - NT * TR       # 72 rows
EPS = 1e-12

NDVE = 3                 # max leading anti chunks per tile on DVE ({0,1})


def _gs_chunks(i):
    """GpSimd-owned trailing anti chunks ({0,1}) for tile i."""
    return (15,)


def _ndve(i):
    """Leading anti chunks on DVE for tile i."""
    return 3 if i % 2 == 0 else 2


def _schedule():
    """(i, k) -> (engine, bank); engine in {'dve','act','gs'}."""
    sched = {}
    nb01 = 0
    nb23 = 0
    for i in range(NT):
        gs = _gs_chunks(i)
        nd = _ndve(i)
        for k in range(TK):
            if k < nd:
                sched[(i, k)] = ("dve", nb01 % 4)
                nb01 += 1
            elif k in gs:
                sched[(i, k)] = ("gs", nb01 % 4)
                nb01 += 1
            else:
                sched[(i, k)] = ("act", 4 + nb23 % 4)
                nb23 += 1
    return sched


_SCHED = _schedule()
# last matmul per bank 1..7 (bank 0 ends with the tail matmul)
_LAST = {}
for (_i, _k), (_e, _b) in _SCHED.items():
    _LAST[_b] = max(_LAST.get(_b, (-1, -1)), (_i, _k))

_CACHE = {}


def _build():
    f32 = mybir.dt.float32
    bf16 = mybir.dt.bfloat16
    Alu = mybir.AluOpType
    Act = mybir.ActivationFunctionType

    nc = bacc.Bacc("TRN2", target_bir_lowering=False, debug=False,
                   num_devices=NCORES)
    yp = nc.dram_tensor("yp", [R, C], f32, kind="ExternalInput")
    yt = nc.dram_tensor("yt", [R], bf16, kind="ExternalInput")
    cm4 = nc.dram_tensor("cm4", [C, 8, C], f32, kind="ExternalOutput")

    with tile.TileContext(nc) as tc:
        with (
            tc.tile_pool(name="const", bufs=1) as cpool,
            tc.tile_pool(name="xin", bufs=4) as xpool,
            tc.tile_pool(name="oh", bufs=5) as ohpool,
            tc.tile_pool(name="an", bufs=5) as anpool,
            tc.tile_pool(name="tree", bufs=4) as tpool,
            tc.tile_pool(name="small", bufs=6) as spool,
            tc.tile_pool(name="psum", bufs=1, space="PSUM") as psum,
        ):
            iota_i = cpool.tile([128, C], mybir.dt.int32)
            nc.gpsimd.iota(iota_i[:], pattern=[[1, C]], base=0,
                           channel_multiplier=0)
            iota_bf = cpool.tile([128, C], bf16)
            nc.vector.tensor_copy(iota_bf[:], iota_i[:])
            # iota_ck[p, c, k] = c  (constant along inner k)
            iota_ck = cpool.tile([128, C, TK], bf16)
            nc.vector.tensor_copy(
                iota_ck[:], iota_bf[:, :, None].broadcast_to([128, C, TK])
            )

            accs = [
                psum.tile([C, C], f32, tag=f"acc{j}", name=f"acc{j}")
                for j in range(8)
            ]
            started = [False] * 8

            def mm(bank, lhsT, rhs, is_last=False):
                nc.tensor.matmul(
                    accs[bank][:], lhsT, rhs,
                    start=not started[bank], stop=is_last,
                )
                started[bank] = True

            def emit_tile(i):
                base = i * TR
                x = xpool.tile([128, TK, C], f32, tag="x", name="x")
                nc.sync.dma_start(
                    x[:],
                    yp.ap()[base : base + TR, :].rearrange(
                        "(p k) c -> p k c", k=TK
                    ),
                )
                t = spool.tile([128, TK], bf16, tag="t", name="t")
                nc.sync.dma_start(
                    t[:],
                    yt.ap()[base : base + TR].rearrange("(p k) -> p k", k=TK),
                )

                # one-hot(true) in ck layout: single 2x is_eq
                oht = ohpool.tile([128, C, TK], bf16, tag="oht", name="oht")
                nc.vector.tensor_tensor(
                    oht[:], iota_ck[:],
                    t[:, None, :].broadcast_to([128, C, TK]),
                    op=Alu.is_equal,
                )

                # rowmax tree; step 1 reads f32, emits RN-bf16
                m1 = tpool.tile([128, TK, 64], bf16, tag="m1", name="m1")
                nc.vector.tensor_tensor(
                    m1[:], x[:, :, 0:64], x[:, :, 64:128], op=Alu.max
                )
                m2 = tpool.tile([128, TK, 32], bf16, tag="m2", name="m2")
                nc.vector.tensor_tensor(
                    m2[:], m1[:, :, 0:32], m1[:, :, 32:64], op=Alu.max
                )
                m3 = tpool.tile([128, TK, 16], bf16, tag="m3", name="m3")
                nc.vector.tensor_tensor(
                    m3[:], m2[:, :, 0:16], m2[:, :, 16:32], op=Alu.max
                )
                m4 = tpool.tile([128, TK, 8], bf16, tag="m4", name="m4")
                nc.vector.tensor_tensor(
                    m4[:], m3[:, :, 0:8], m3[:, :, 8:16], op=Alu.max
                )
                rmax = spool.tile([128, TK], f32, tag="rmax", name="rmax")
                nc.vector.tensor_reduce(
                    rmax[:], m4[:], axis=mybir.AxisListType.X, op=Alu.max
                )
                # threshold strictly between true rowmax and its RN-bf16
                # neighborhood: thr = rmax * (1 - 2^-8); rowmax > 1 here
                thr = spool.tile([128, TK], f32, tag="thr", name="thr")
                nc.vector.tensor_scalar(
                    thr[:], rmax[:], float(1.0 - 2.0 ** -8), None,
                    op0=Alu.mult,
                )

                # anti chunks
                anti = anpool.tile([128, TK, C], bf16, tag="anti",
                                   name="anti")
                nd = _ndve(i)
                nc.vector.tensor_tensor(
                    anti[:, 0:nd, :], x[:, 0:nd, :],
                    thr[:, 0:nd, None].broadcast_to([128, nd, C]),
                    op=Alu.is_lt,
                )
                gs = _gs_chunks(i)
                for k in range(nd, TK):
                    if k in gs:
                        nc.gpsimd.tensor_scalar(
                            anti[:, k, :], x[:, k, :], thr[:, k : k + 1],
                            None, op0=Alu.is_lt,
                        )
                    else:
                        # sign(rmax - s*x) with s = 1/(1-2^-8): same
                        # threshold folded into the activation scale
                        nc.scalar.activation(
                            anti[:, k, :], x[:, k, :], Act.Sign,
                            bias=rmax[:, k : k + 1],
                            scale=float(-1.0 / (1.0 - 2.0 ** -8)),
                        )

                for k in range(TK):
                    eng, bank = _SCHED[(i, k)]
                    mm(bank, oht[:, :, k], anti[:, k, :],
                       is_last=(bank != 0 and _LAST[bank] == (i, k)))

            for i in range(NT):
                emit_tile(i)

            # tail rows (72): {0,1} semantics -> bank 0
            base = NT * TR
            xt = xpool.tile([TAIL, 1, C], f32, tag="xtail", name="xt")
            nc.sync.dma_start(
                xt[:],
                yp.ap()[base : R, :].rearrange("(p k) c -> p k c", k=1),
            )
            tt = spool.tile([TAIL, 1], bf16, tag="ttail", name="tt")
            nc.sync.dma_start(
                tt[:], yt.ap()[base : R].rearrange("(p k) -> p k", k=1)
            )
            tt_f = spool.tile([TAIL, 1], f32, tag="ttailf", name="tt_f")
            nc.vector.tensor_copy(tt_f[:], tt[:])
            rmax_t = spool.tile([TAIL, 1], f32, tag="rmaxtail", name="rmax_t")
            nc.vector.tensor_reduce(
                rmax_t[:], xt[:], axis=mybir.AxisListType.X, op=Alu.max
            )
            anti_t = anpool.tile([TAIL, C], bf16, tag="antitail",
                                 name="anti_t")
            oht_t = ohpool.tile([TAIL, C], bf16, tag="ohttail", name="oht_t")
            nc.vector.tensor_scalar(
                anti_t[:], xt[:, 0, :], rmax_t[:], None, op0=Alu.is_lt
            )
            nc.vector.tensor_scalar(
                oht_t[:], iota_bf[:TAIL, :], tt_f[:], None, op0=Alu.is_equal
            )
            nc.tensor.matmul(
                accs[0][:], oht_t[:], anti_t[:], start=False, stop=True
            )

            out_sb = cpool.tile([C, 8, C], f32)
            for j in range(8):
                nc.scalar.copy(out_sb[:, j, :], accs[j][:])
            nc.sync.dma_start(cm4.ap()[:], out_sb[:])

    nc.compile()
    return nc


def _group01_mask():
    """Per-core mask over R rows: True if the row's anti chunk has {0,1}
    semantics (DVE/GpSimd/tail), False for ACT ({-1,+1})."""
    r = np.arange(R)
    i = r // TR
    k = r % TK
    m = i == NT  # tail rows
    for it in range(NT):
        sel = i == it
        m |= sel & (k < _ndve(it))
        gs = _gs_chunks(it)
        if gs:
            m |= sel & np.isin(k, gs)
    return m


_G01 = _group01_mask()


def _get_nc():
    if "nc" not in _CACHE:
        _CACHE["nc"] = _build()
    return _CACHE["nc"]


def _run(y_pred, y_true, trace=False):
    import ml_dtypes

    nc = _get_nc()
    y_pred = np.ascontiguousarray(np.asarray(y_pred, dtype=np.float32))
    yt_i = np.asarray(y_true).astype(np.int64)
    yt_bf = yt_i.astype(ml_dtypes.bfloat16)
    in_maps = [
        {
            "yp": y_pred[c * R : (c + 1) * R],
            "yt": np.ascontiguousarray(yt_bf[c * R : (c + 1) * R]),
        }
        for c in range(NCORES)
    ]
    res = None
    for attempt in range(3):
        try:
            res = bass_utils.run_bass_kernel_spmd(
                nc, in_maps, core_ids=list(range(NCORES)), trace=trace
            )
            break
        except Exception:
            if attempt == 2:
                raise
            time.sleep(2.0)
    b01 = np.zeros((C, C), dtype=np.float64)
    b23 = np.zeros((C, C), dtype=np.float64)
    s01 = np.zeros(C, dtype=np.float64)
    s23 = np.zeros(C, dtype=np.float64)
    for c, r in enumerate(res.results):
        cm_b = r["cm4"].astype(np.float64)
        b01 += cm_b[:, 0:4, :].sum(axis=1)
        b23 += cm_b[:, 4:8, :].sum(axis=1)
        yt_c = yt_i[c * R : (c + 1) * R]
        s01 += np.bincount(yt_c[_G01], minlength=C)
        s23 += np.bincount(yt_c[~_G01], minlength=C)
    cm = (s01[:, None] - b01) + (s23[:, None] - b23) / 2.0
    diag = np.diagonal(cm)
    precision = diag / (cm.sum(axis=1) + EPS)
    recall = diag / (cm.sum(axis=0) + EPS)
    f1 = 2.0 * precision * recall / (precision + recall + EPS)
    return np.float32(f1.mean()), res


def kernel(y_pred, y_true):
    out, _ = _run(y_pred, y_true, trace=False)
    return out


# revision 6
# speedup vs baseline: 1.1802x; 1.1600x over previous
"""F1-score (macro) kernel for Trainium2, 8 NeuronCores.

Data-parallel over rows (125000/core). Per tile of 2048 rows
([128p, TK=16, 128c], row = base + p*TK + k):

  - DVE : rowmax via pairwise-max tree. Step 1 reads the HIGH bf16 halves
          of the f32 tile (bitcast stride-2 view) -> truncated-bf16 max
          (trunc <= true max, so the argmax position always compares as
          "not less"). Steps 2..4 run at DVE 2x on packed bf16; final
          8-wide reduce emits f32.
  - DVE : oht_ck[p,c,k] = (c == y_true) in ck-layout - a single is_eq at
          DVE 2x (broadcasting t along the MIDDLE dim keeps every
          operand's last dim packed bf16).
  - anti split: first NDVE chunks as ONE sub-tile tensor_tensor is_lt on
          DVE ({0,1}); ~half the remaining on ACT via Sign ({-1,+1}!);
          one trailing chunk on GpSimd every other tile ({0,1}).
  - PE  : acc[bank] += oht_ck[:,:,k]^T @ anti[:,k,:] (bf16, 4 PSUM banks:
          banks 0/1 collect {0,1}-chunks, banks 2/3 the {-1,+1} chunks).

Host: with per-group supports S01/S23 (row->engine map is static):
  pred01 = S01 - (bank0+bank1),  pred23 = (S23 - (bank2+bank3)) / 2
  cm = pred01 + pred23;  macro-F1 epilogue on [128,128].

bf16 truncation ties perturb F1 by ~2.7e-4 (measured in numpy on the
actual inputs), far under the 2e-2 gate.
"""

import sys
import time

if "/opt/trn_rl_repo" not in sys.path:
    sys.path.insert(0, "/opt/trn_rl_repo")

import numpy as np

import concourse.bacc as bacc
import concourse.mybir as mybir
import concourse.tile as tile
from concourse import bass_utils

C = 128
N = 1_000_000
NCORES = 8
R = N // NCORES          # 125000 rows per core
TK = 16                  # chunks (of 128 rows) per tile
TR = 128 * TK            # 2048 rows per tile
NT = R // TR             # 61 tiles
TAIL = R - NT * TR       # 72 rows
EPS = 1e-12

NDVE = 3                 # max leading anti chunks per tile on DVE ({0,1})


def _gs_chunks(i):
    """GpSimd-owned trailing anti chunks ({0,1}) for tile i."""
    return (15,)


def _ndve(i):
    """Leading anti chunks on DVE for tile i."""
    return 3 if i % 2 == 0 else 2


def _schedule():
    """(i, k) -> (engine, bank); engine in {'dve','act','gs'}."""
    sched = {}
    nb01 = 0
    nb23 = 0
    for i in range(NT):
        gs = _gs_chunks(i)
        nd = _ndve(i)
        for k in range(TK):
            if k < nd:
                sched[(i, k)] = ("dve", nb01 % 4)
                nb01 += 1
            elif k in gs:
                sched[(i, k)] = ("gs", nb01 % 4)
                nb01 += 1
            else:
                sched[(i, k)] = ("act", 4 + nb23 % 4)
                nb23 += 1
    return sched


_SCHED = _schedule()
# last matmul per bank 1..7 (bank 0 ends with the tail matmul)
_LAST = {}
for (_i, _k), (_e, _b) in _SCHED.items():
    _LAST[_b] = max(_LAST.get(_b, (-1, -1)), (_i, _k))

_CACHE = {}


def _build():
    f32 = mybir.dt.float32
    bf16 = mybir.dt.bfloat16
    Alu = mybir.AluOpType
    Act = mybir.ActivationFunctionType

    nc = bacc.Bacc("TRN2", target_bir_lowering=False, debug=False,
                   num_devices=NCORES)
    yp = nc.dram_tensor("yp", [R, C], f32, kind="ExternalInput")
    yt = nc.dram_tensor("yt", [R], bf16, kind="ExternalInput")
    cm4 = nc.dram_tensor("cm4", [C, 8, C], f32, kind="ExternalOutput")

    with tile.TileContext(nc) as tc:
        with (
            tc.tile_pool(name="const", bufs=1) as cpool,
            tc.tile_pool(name="xin", bufs=4) as xpool,
            tc.tile_pool(name="oh", bufs=5) as ohpool,
            tc.tile_pool(name="an", bufs=5) as anpool,
            tc.tile_pool(name="tree", bufs=4) as tpool,
            tc.tile_pool(name="small", bufs=6) as spool,
            tc.tile_pool(name="psum", bufs=1, space="PSUM") as psum,
        ):
            iota_i = cpool.tile([128, C], mybir.dt.int32)
            nc.gpsimd.iota(iota_i[:], pattern=[[1, C]], base=0,
                           channel_multiplier=0)
            iota_bf = cpool.tile([128, C], bf16)
            nc.vector.tensor_copy(iota_bf[:], iota_i[:])
            # iota_ck[p, c, k] = c  (constant along inner k)
            iota_ck = cpool.tile([128, C, TK], bf16)
            nc.vector.tensor_copy(
                iota_ck[:], iota_bf[:, :, None].broadcast_to([128, C, TK])
            )

            accs = [
                psum.tile([C, C], f32, tag=f"acc{j}", name=f"acc{j}")
                for j in range(8)
            ]
            started = [False] * 8

            def mm(bank, lhsT, rhs, is_last=False):
                nc.tensor.matmul(
                    accs[bank][:], lhsT, rhs,
                    start=not started[bank], stop=is_last,
                )
                started[bank] = True

            def emit_tile(i):
                base = i * TR
                x = xpool.tile([128, TK, C], f32, tag="x", name="x")
                nc.sync.dma_start(
                    x[:],
                    yp.ap()[base : base + TR, :].rearrange(
                        "(p k) c -> p k c", k=TK
                    ),
                )
                t = spool.tile([128, TK], bf16, tag="t", name="t")
                nc.sync.dma_start(
                    t[:],
                    yt.ap()[base : base + TR].rearrange("(p k) -> p k", k=TK),
                )

                # one-hot(true) in ck layout: single 2x is_eq
                oht = ohpool.tile([128, C, TK], bf16, tag="oht", name="oht")
                nc.vector.tensor_tensor(
                    oht[:], iota_ck[:],
                    t[:, None, :].broadcast_to([128, C, TK]),
                    op=Alu.is_equal,
                )

                # exact f32 rowmax (reduce is input-count-bound; no tree
                # variant beats it on this hardware)
                rmax = spool.tile([128, TK], f32, tag="rmax", name="rmax")
                nc.vector.tensor_reduce(
                    rmax[:], x[:], axis=mybir.AxisListType.X, op=Alu.max
                )

                # anti chunks
                anti = anpool.tile([128, TK, C], bf16, tag="anti",
                                   name="anti")
                nd = _ndve(i)
                nc.vector.tensor_tensor(
                    anti[:, 0:nd, :], x[:, 0:nd, :],
                    rmax[:, 0:nd, None].broadcast_to([128, nd, C]),
                    op=Alu.is_lt,
                )
                gs = _gs_chunks(i)
                for k in range(nd, TK):
                    if k in gs:
                        nc.gpsimd.tensor_scalar(
                            anti[:, k, :], x[:, k, :], rmax[:, k : k + 1],
                            None, op0=Alu.is_lt,
                        )
                    else:
                        # sign(rmax - x) in {0,1}: x <= rmax always (exact)
                        nc.scalar.activation(
                            anti[:, k, :], x[:, k, :], Act.Sign,
                            bias=rmax[:, k : k + 1], scale=-1.0,
                        )

                for k in range(TK):
                    eng, bank = _SCHED[(i, k)]
                    mm(bank, oht[:, :, k], anti[:, k, :],
                       is_last=(bank != 0 and _LAST[bank] == (i, k)))

            for i in range(NT):
                emit_tile(i)

            # tail rows (72): {0,1} semantics -> bank 0
            base = NT * TR
            xt = xpool.tile([TAIL, 1, C], f32, tag="xtail", name="xt")
            nc.sync.dma_start(
                xt[:],
                yp.ap()[base : R, :].rearrange("(p k) c -> p k c", k=1),
            )
            tt = spool.tile([TAIL, 1], bf16, tag="ttail", name="tt")
            nc.sync.dma_start(
                tt[:], yt.ap()[base : R].rearrange("(p k) -> p k", k=1)
            )
            tt_f = spool.tile([TAIL, 1], f32, tag="ttailf", name="tt_f")
            nc.vector.tensor_copy(tt_f[:], tt[:])
            rmax_t = spool.tile([TAIL, 1], f32, tag="rmaxtail", name="rmax_t")
            nc.vector.tensor_reduce(
                rmax_t[:], xt[:], axis=mybir.AxisListType.X, op=Alu.max
            )
            anti_t = anpool.tile([TAIL, C], bf16, tag="antitail",
                                 name="anti_t")
            oht_t = ohpool.tile([TAIL, C], bf16, tag="ohttail", name="oht_t")
            nc.vector.tensor_scalar(
                anti_t[:], xt[:, 0, :], rmax_t[:], None, op0=Alu.is_lt
            )
            nc.vector.tensor_scalar(
                oht_t[:], iota_bf[:TAIL, :], tt_f[:], None, op0=Alu.is_equal
            )
            nc.tensor.matmul(
                accs[0][:], oht_t[:], anti_t[:], start=False, stop=True
            )

            out_sb = cpool.tile([C, 8, C], f32)
            for j in range(8):
                nc.scalar.copy(out_sb[:, j, :], accs[j][:])
            nc.sync.dma_start(cm4.ap()[:], out_sb[:])

    nc.compile()
    return nc


def _group01_mask():
    """Per-core mask over R rows: True if the row's anti chunk has {0,1}
    semantics (DVE/GpSimd/tail), False for ACT ({-1,+1})."""
    r = np.arange(R)
    i = r // TR
    k = r % TK
    m = i == NT  # tail rows
    for it in range(NT):
        sel = i == it
        m |= sel & (k < _ndve(it))
        gs = _gs_chunks(it)
        if gs:
            m |= sel & np.isin(k, gs)
    return m


_G01 = _group01_mask()


def _get_nc():
    if "nc" not in _CACHE:
        _CACHE["nc"] = _build()
    return _CACHE["nc"]


def _run(y_pred, y_true, trace=False):
    import ml_dtypes

    nc = _get_nc()
    y_pred = np.ascontiguousarray(np.asarray(y_pred, dtype=np.float32))
    yt_i = np.asarray(y_true).astype(np.int64)
    yt_bf = yt_i.astype(ml_dtypes.bfloat16)
    in_maps = [
        {
            "yp": y_pred[c * R : (c + 1) * R],
            "yt": np.ascontiguousarray(yt_bf[c * R : (c + 1) * R]),
        }
        for c in range(NCORES)
    ]
    res = None
    for attempt in range(3):
        try:
            res = bass_utils.run_bass_kernel_spmd(
                nc, in_maps, core_ids=list(range(NCORES)), trace=trace
            )
            break
        except Exception:
            if attempt == 2:
                raise
            time.sleep(2.0)
    cm_dev = np.zeros((C, C), dtype=np.float64)
    for r in res.results:
        cm_dev += r["cm4"].astype(np.float64).sum(axis=1)
    support = np.bincount(yt_i, minlength=C).astype(np.float64)
    cm = support[:, None] - cm_dev
    diag = np.diagonal(cm)
    precision = diag / (cm.sum(axis=1) + EPS)
    recall = diag / (cm.sum(axis=0) + EPS)
    f1 = 2.0 * precision * recall / (precision + recall + EPS)
    return np.float32(f1.mean()), res


def kernel(y_pred, y_true):
    out, _ = _run(y_pred, y_true, trace=False)
    return out
